# revision 22
# baseline (speedup 1.0000x reference)
"""Distributed causal attention head for TRN2 (8 NeuronCores), v6.

Problem: B=4, S=4096, D=1024, H=64 fp32.
  q,k,v = x @ W{q,k,v}; scores = q k^T / sqrt(H); causal softmax; out = P v.

Sharding (fully SPMD-uniform, one NEFF, NO collectives):
  - 4 batches x 2 cores per batch. Within a pair the KEY dimension is split
    by interleaved 128-row chunks: core g owns global key chunks {2i+g}.
  - Each core loads the FULL batch x^T [1024, 4096] bf16 (host pre-transposed)
    with the 128-col chunks PAIR-SWAP permuted for g=1, so own key chunks sit
    at even slots on every core -> all slicing is core-independent. Masks are
    built in permuted query order per core; the host un-permutes the g=1
    output columns, merges the pair (add), divides, and transposes.
  - Every measured collective (barrier/AllGather) could not execute before
    ~50-60us into the kernel, so q is simply computed from the full x instead
    of being gathered: the extra projection work hides under the x DMA.

Compute layout:
  - k|q packed projection (wkq = [Wk|Wq] -> k in psum rows 0:64, q in 64:128)
    processed in 512-col eighths, each immediately followed by its attention
    q-block so the strict-FIFO PE never waits on not-yet-loaded data.
  - V^T for own chunks via strided N=256 matmuls (wv stationary), transposed
    into natural layout with the DMA-xbar transpose engine (no PE transposes).
  - Scores transposed with 2x PE row tiling (64-contract): tile (0,0) does
    local chunks 0..t of q-block t, tile (64,0) chunks t+1..2t+1. kT/qT are
    duplicated into both SBUF partition halves via SBUF->SBUF DMA on the
    ScalarE DMA rings (separate from the bulk x loads on the SP rings).
  - Each score slot = one 2-bank PSUM set (2 chunks). exp: ScalarE for the
    T0 chunk; the T8 chunk uses a one-op DVE Schraudolph bit-trick
    (bf16 bits = uint16(score * A + B)) on 3 of every 4 sets, balancing
    ScalarE and VectorE. V is augmented with a ones column so the AV matmul
    also produces the softmax denominator (row 64 of [65, 512]).
"""

import sys

sys.path.insert(0, "/opt/trn_rl_repo")

import numpy as np
import ml_dtypes

B, S, D, H = 4, 4096, 1024, 64
QB = 512
NQB = S // QB           # 8 query blocks / projection eighths
NKC = S // 256          # 16 own key chunks per core
BF16 = ml_dtypes.bfloat16

_CACHE = {}


def _build():
    import concourse.bass as bass
    import concourse.mybir as mybir
    from concourse import bacc, tile
    from concourse.bass import ts

    f32 = mybir.dt.float32
    bf16 = mybir.dt.bfloat16
    u16 = mybir.dt.uint16
    Alu = mybir.AluOpType
    Act = mybir.ActivationFunctionType

    # Schraudolph exp-approximation constants for the DVE path:
    # bf16 bits of exp(s/8) ~= uint16(s * SCH_A + SCH_B)
    SCH_C = 486411
    SCH_A = 0.125 * float(1 << 23) / float(np.log(2.0)) / 65536.0
    SCH_B = float((127 << 23) - SCH_C) / 65536.0

    nc = bacc.Bacc(None, target_bir_lowering=False)

    x_ext = nc.declare_dram_parameter("x", [D, S], bf16, isOutput=False)
    wkq_ext = nc.declare_dram_parameter("wkq", [128, 8 * 128], bf16, isOutput=False)
    wv_ext = nc.declare_dram_parameter("wv", [128, 8 * H], bf16, isOutput=False)
    mask_ext = nc.declare_dram_parameter("mask", [128, 2 * QB], bf16, isOutput=False)
    out_ext = nc.declare_dram_parameter("out", [H + 1, S], f32, isOutput=True)

    with tile.TileContext(nc) as tc:
        with (
            tc.tile_pool(name="persist", bufs=1) as persist,
            tc.tile_pool(name="dram", bufs=1, space="DRAM") as dram,
        ):
            # --- persistent SBUF tensors ---
            xT = persist.tile([128, 8, S // 256, 2, 128], bf16, tag="xT")
            wkq_sb = persist.tile([128, 8, 128], bf16, tag="wkq")
            wv_sb = persist.tile([128, 8, H], bf16, tag="wv")
            mask_sb = persist.tile([128, 2, QB], bf16, tag="mask")
            kT2 = persist.tile([128, NKC, 128], bf16, tag="kT2")
            qT2 = persist.tile([128, S], bf16, tag="qT2")
            v_all = persist.tile([128, NKC, H + 2], bf16, tag="v_all")
            zjunk = persist.tile([128, 8], f32, tag="zjunk")
            ejunk = persist.tile([128, 8], bf16, tag="ejunk")

            # preload the exp activation table set early (it costs ~2.7us)
            nc.vector.memset(zjunk[:], 0.0)
            nc.scalar.activation(ejunk[:], zjunk[:], Act.Exp)
            nc.vector.memset(v_all[:, :, H], 1.0)

            # small contiguous weight/mask loads, then bulk x per column quarter
            nc.sync.dma_start(out=wkq_sb[:], in_=wkq_ext[:])
            nc.sync.dma_start(out=wv_sb[:], in_=wv_ext[:])
            nc.sync.dma_start(out=mask_sb[:], in_=mask_ext[:])
            for qt in range(4):
                for dc in range(8):
                    nc.sync.dma_start(
                        out=xT[:, dc, 4 * qt : 4 * (qt + 1), :, :],
                        in_=x_ext[ts(dc, 128), ts(qt, S // 4)],
                    )

            with (
                tc.tile_pool(name="pj", bufs=2, space="PSUM") as pj_pool,
                tc.tile_pool(name="pv", bufs=1, space="PSUM") as pv_pool,
                tc.tile_pool(name="st", bufs=3, space="PSUM") as st_pool,
                tc.tile_pool(name="av", bufs=2, space="PSUM") as av_pool,
                tc.tile_pool(name="p", bufs=6) as p_pool,
                tc.tile_pool(name="o", bufs=3) as o_pool,
            ):
                set_idx = 0


                for t in range(NQB):
                    # ---- projection eighth t: x cols [512t, 512t+512) ----
                    # kq (N=512) and the two natural-V chunk matmuls (N=64)
                    # interleave so the long kq matmuls hide the V LDWEIGHTS
                    kq_ps = pj_pool.tile([128, QB], f32, tag="kq")
                    v_ps = pv_pool.tile([128, 2, H], f32, tag="v")
                    for dc in range(8):
                        nc.tensor.matmul(
                            kq_ps[:],
                            lhsT=wkq_sb[:, dc, :],
                            rhs=xT[:, dc, 2 * t : 2 * t + 2, :, :],
                            start=(dc == 0),
                            stop=(dc == 7),
                        )
                        for j in range(2):
                            nc.tensor.matmul(
                                v_ps[:, j, :],
                                lhsT=xT[:, dc, 2 * t + j, 0, :],
                                rhs=wv_sb[:, dc, :],
                                start=(dc == 0 and j == 0),
                                stop=(dc == 7),
                                skip_group_check=True,
                            )
                    for j in range(2):
                        nc.any.tensor_copy(v_all[:, 2 * t + j, 0:H], v_ps[:, j, :])
                    # q: psum rows 64:128 -> qT2 high half, then low-half dup
                    nc.vector.tensor_copy(qT2[64:128, ts(t, QB)], kq_ps[64:128, :])
                    nc.scalar.dma_start(
                        out=qT2[0:64, ts(t, QB)], in_=qT2[64:128, ts(t, QB)]
                    )
                    # k: own chunks sit at even slots (4t, 4t+2)
                    for j in range(2):
                        nc.any.tensor_copy(
                            kT2[0:64, 2 * t + j, :],
                            kq_ps[0:64, 2 * j * 128 : (2 * j + 1) * 128],
                        )
                    nc.scalar.dma_start(
                        out=kT2[64:128, 2 * t : 2 * t + 2, :],
                        in_=kT2[0:64, 2 * t : 2 * t + 2, :],
                    )
                    # ---- attention q-block t ----
                    E = 2 * (t + 1)
                    av = av_pool.tile([H + 1, QB], f32, tag="av")
                    for s in range(t + 1):
                        c0, c1 = s, t + 1 + s
                        st0 = st_pool.tile([128, QB], f32, tag="st")
                        st1 = st_pool.tile([128, QB], f32, tag="st")
                        p = p_pool.tile([128, 2, QB], bf16, tag="p")
                        for hh, cid, stt in ((0, c0, st0), (1, c1, st1)):
                            nc.tensor.matmul(
                                stt[:],
                                lhsT=kT2[64 * hh : 64 * hh + 64, cid, :],
                                rhs=qT2[64 * hh : 64 * hh + 64, ts(t, QB)],
                                start=True,
                                stop=True,
                                tile_position=(64 * hh, 0),
                            )
                        # T0 chunk on ScalarE; T8 chunk on the DVE bit-trick,
                        # except every 6th slot (keeps the engines balanced)
                        nc.scalar.activation(
                            p[:, 0, :], st0[:], Act.Exp, scale=0.125
                        )
                        if set_idx % 6 == 5:
                            nc.scalar.activation(
                                p[:, 1, :], st1[:], Act.Exp, scale=0.125
                            )
                        else:
                            nc.vector.tensor_scalar(
                                p[:, 1, :].bitcast(u16),
                                st1[:],
                                SCH_A,
                                SCH_B,
                                Alu.mult,
                                Alu.add,
                            )
                        set_idx += 1
                        for hh, cid in ((0, c0), (1, c1)):
                            if cid >= E - 2:
                                nc.vector.tensor_tensor(
                                    p[:, hh, :],
                                    p[:, hh, :],
                                    mask_sb[:, cid - (E - 2), :],
                                    Alu.mult,
                                )
                        for hh, cid in ((0, c0), (1, c1)):
                            nc.tensor.matmul(
                                av[:],
                                lhsT=v_all[:, cid, 0 : H + 1],
                                rhs=p[:, hh, :],
                                start=(cid == 0),
                                stop=(cid == E - 1),
                            )
                    o = o_pool.tile([H + 1, QB], f32, tag="o")
                    nc.vector.tensor_copy(o[:], av[:])
                    nc.sync.dma_start(out=out_ext[:, ts(t, QB)], in_=o[:])

    nc.finalize()
    return nc


def _make_masks(g: int) -> np.ndarray:
    # mask[j][kk, qq]: qq is the PERMUTED block-local query col; its global
    # offset inside the block is qq_g. Key row kk belongs to own chunk with
    # in-block global offset 128*(2j+g).
    m = np.zeros((2, 128, QB), dtype=np.float32)
    qq = np.arange(QB)
    chunk4 = qq // 128
    if g == 1:
        chunk4 = chunk4 ^ 1
    qq_g = chunk4 * 128 + (qq % 128)
    for j in range(2):
        dk = 128 * (2 * j + g) + np.arange(128)[:, None]
        m[j] = (qq_g[None, :] >= dk).astype(np.float32)
    return m.astype(BF16)


def _perm_cols(a: np.ndarray, g: int) -> np.ndarray:
    """Pair-swap 128-col chunks along the last axis when g=1 (involution)."""
    if g == 0:
        return a
    shp = a.shape
    v = a.reshape(shp[:-1] + (shp[-1] // 256, 2, 128))
    return np.ascontiguousarray(v[..., ::-1, :].reshape(shp))


def _shard_inputs(input, Wq, Wk, Wv):
    x = np.asarray(input)
    wkq = np.concatenate([Wk, Wq], axis=1).astype(np.float32)  # [D, 128]
    wkq_h = np.ascontiguousarray(
        wkq.reshape(8, 128, 128).transpose(1, 0, 2).reshape(128, 8 * 128)
    ).astype(BF16)
    wv_h = np.ascontiguousarray(
        np.asarray(Wv, dtype=np.float32)
        .reshape(8, 128, H)
        .transpose(1, 0, 2)
        .reshape(128, 8 * H)
    ).astype(BF16)
    masks = []
    for g in range(2):
        m = _make_masks(g)  # [2, 128, QB]
        masks.append(np.ascontiguousarray(m.transpose(1, 0, 2).reshape(128, 2 * QB)))
    in_maps = []
    xTs = {}
    for b in range(B):
        xT = np.ascontiguousarray(x[b].T).astype(BF16)  # [D, S] global cols
        xTs[(b, 0)] = xT
        xTs[(b, 1)] = _perm_cols(xT, 1)
    for c in range(8):
        b, g = c // 2, c % 2
        in_maps.append(
            {"x": xTs[(b, g)], "wkq": wkq_h, "wv": wv_h, "mask": masks[g]}
        )
    return in_maps


def _unshard(results):
    out = np.empty((B, S, H), dtype=np.float32)
    for b in range(B):
        merged = results[2 * b]["out"] + _perm_cols(results[2 * b + 1]["out"], 1)
        out[b] = (merged[:H] / merged[H : H + 1]).T
    return out


def _run(inputs, trace=False):
    from concourse.bass_utils import run_bass_kernel_spmd

    if "nc" not in _CACHE:
        _CACHE["nc"] = _build()
    nc = _CACHE["nc"]
    in_maps = _shard_inputs(**inputs)
    res = run_bass_kernel_spmd(nc, in_maps, core_ids=list(range(8)), trace=trace)
    out = _unshard(res.results)
    return out, res


def kernel(**inputs) -> np.ndarray:
    out, _ = _run(inputs, trace=False)
    return out


# revision 23
# speedup vs baseline: 1.1755x; 1.1755x over previous
"""Distributed causal attention head for TRN2 (8 NeuronCores), v6.

Problem: B=4, S=4096, D=1024, H=64 fp32.
  q,k,v = x @ W{q,k,v}; scores = q k^T / sqrt(H); causal softmax; out = P v.

Sharding (fully SPMD-uniform, one NEFF, NO collectives):
  - 4 batches x 2 cores per batch. Within a pair the KEY dimension is split
    by interleaved 128-row chunks: core g owns global key chunks {2i+g}.
  - Each core loads the FULL batch x^T [1024, 4096] bf16 (host pre-transposed)
    with the 128-col chunks PAIR-SWAP permuted for g=1, so own key chunks sit
    at even slots on every core -> all slicing is core-independent. Masks are
    built in permuted query order per core; the host un-permutes the g=1
    output columns, merges the pair (add), divides, and transposes.
  - Every measured collective (barrier/AllGather) could not execute before
    ~50-60us into the kernel, so q is simply computed from the full x instead
    of being gathered: the extra projection work hides under the x DMA.

Compute layout:
  - k|q packed projection (wkq = [Wk|Wq] -> k in psum rows 0:64, q in 64:128)
    processed in 512-col eighths, each immediately followed by its attention
    q-block so the strict-FIFO PE never waits on not-yet-loaded data.
  - V^T for own chunks via strided N=256 matmuls (wv stationary), transposed
    into natural layout with the DMA-xbar transpose engine (no PE transposes).
  - Scores transposed with 2x PE row tiling (64-contract): tile (0,0) does
    local chunks 0..t of q-block t, tile (64,0) chunks t+1..2t+1. kT/qT are
    duplicated into both SBUF partition halves via SBUF->SBUF DMA on the
    ScalarE DMA rings (separate from the bulk x loads on the SP rings).
  - Each score slot = one 2-bank PSUM set (2 chunks). exp: ScalarE for the
    T0 chunk; the T8 chunk uses a one-op DVE Schraudolph bit-trick
    (bf16 bits = uint16(score * A + B)) on 3 of every 4 sets, balancing
    ScalarE and VectorE. V is augmented with a ones column so the AV matmul
    also produces the softmax denominator (row 64 of [65, 512]).
"""

import sys

sys.path.insert(0, "/opt/trn_rl_repo")

import numpy as np
import ml_dtypes

B, S, D, H = 4, 4096, 1024, 64
QB = 512
NQB = S // QB           # 8 query blocks / projection eighths
NKC = S // 256          # 16 own key chunks per core
BF16 = ml_dtypes.bfloat16

_CACHE = {}


def _build():
    import concourse.bass as bass
    import concourse.mybir as mybir
    from concourse import bacc, tile
    from concourse.bass import ts

    f32 = mybir.dt.float32
    bf16 = mybir.dt.bfloat16
    u16 = mybir.dt.uint16
    Alu = mybir.AluOpType
    Act = mybir.ActivationFunctionType

    # Schraudolph exp-approximation constants for the DVE path:
    # bf16 bits of exp(s/8) ~= uint16(s * SCH_A + SCH_B)
    SCH_C = 486411
    SCH_A = 0.125 * float(1 << 23) / float(np.log(2.0)) / 65536.0
    SCH_B = float((127 << 23) - SCH_C) / 65536.0

    nc = bacc.Bacc(None, target_bir_lowering=False)

    x_ext = nc.declare_dram_parameter("x", [D, S], bf16, isOutput=False)
    wkq_ext = nc.declare_dram_parameter("wkq", [128, 8 * 128], bf16, isOutput=False)
    wv_ext = nc.declare_dram_parameter("wv", [128, 8 * H], bf16, isOutput=False)
    mask_ext = nc.declare_dram_parameter("mask", [128, 2 * QB], bf16, isOutput=False)
    out_ext = nc.declare_dram_parameter("out", [H + 1, S], f32, isOutput=True)

    with tile.TileContext(nc) as tc:
        with (
            tc.tile_pool(name="persist", bufs=1) as persist,
            tc.tile_pool(name="dram", bufs=1, space="DRAM") as dram,
        ):
            # --- persistent SBUF tensors ---
            xT = persist.tile([128, 8, S // 256, 2, 128], bf16, tag="xT")
            wkq_sb = persist.tile([128, 8, 128], bf16, tag="wkq")
            wv_sb = persist.tile([128, 8, H], bf16, tag="wv")
            mask_sb = persist.tile([128, 2, QB], bf16, tag="mask")
            kT2 = persist.tile([128, NKC, 128], bf16, tag="kT2")
            qT2 = persist.tile([128, S], bf16, tag="qT2")
            v_all = persist.tile([128, NKC, H + 2], bf16, tag="v_all")
            zjunk = persist.tile([128, 8], f32, tag="zjunk")
            ejunk = persist.tile([128, 8], bf16, tag="ejunk")

            # preload the exp activation table set early (it costs ~2.7us)
            nc.vector.memset(zjunk[:], 0.0)
            nc.scalar.activation(ejunk[:], zjunk[:], Act.Exp)
            nc.vector.memset(v_all[:, :, H], 1.0)

            # small contiguous weight/mask loads, then bulk x per column quarter
            nc.sync.dma_start(out=wkq_sb[:], in_=wkq_ext[:])
            nc.sync.dma_start(out=wv_sb[:], in_=wv_ext[:])
            nc.sync.dma_start(out=mask_sb[:], in_=mask_ext[:])
            for qt in range(4):
                for dc in range(8):
                    nc.sync.dma_start(
                        out=xT[:, dc, 4 * qt : 4 * (qt + 1), :, :],
                        in_=x_ext[ts(dc, 128), ts(qt, S // 4)],
                    )

            with (
                tc.tile_pool(name="pj", bufs=2, space="PSUM") as pj_pool,
                tc.tile_pool(name="pv", bufs=1, space="PSUM") as pv_pool,
                tc.tile_pool(name="st", bufs=3, space="PSUM") as st_pool,
                tc.tile_pool(name="av", bufs=2, space="PSUM") as av_pool,
                tc.tile_pool(name="p", bufs=6) as p_pool,
                tc.tile_pool(name="o", bufs=3) as o_pool,
            ):
                set_idx = 0


                for t in range(NQB):
                    # ---- projection eighth t: x cols [512t, 512t+512) ----
                    kq_ps = pj_pool.tile([128, QB], f32, tag="kq")
                    for dc in range(8):
                        nc.tensor.matmul(
                            kq_ps[:],
                            lhsT=wkq_sb[:, dc, :],
                            rhs=xT[:, dc, 2 * t : 2 * t + 2, :, :],
                            start=(dc == 0),
                            stop=(dc == 7),
                        )
                    # natural-layout V for the two own chunks (x^T stationary)
                    for j in range(2):
                        v_ps = pv_pool.tile([128, H], f32, tag="v")
                        for dc in range(8):
                            nc.tensor.matmul(
                                v_ps[:],
                                lhsT=xT[:, dc, 2 * t + j, 0, :],
                                rhs=wv_sb[:, dc, :],
                                start=(dc == 0),
                                stop=(dc == 7),
                            )
                        nc.any.tensor_copy(v_all[:, 2 * t + j, 0:H], v_ps[:])
                    # q: psum rows 64:128 -> qT2 high half, then low-half dup
                    nc.vector.tensor_copy(qT2[64:128, ts(t, QB)], kq_ps[64:128, :])
                    nc.scalar.dma_start(
                        out=qT2[0:64, ts(t, QB)], in_=qT2[64:128, ts(t, QB)]
                    )
                    # k: own chunks sit at even slots (4t, 4t+2)
                    for j in range(2):
                        nc.any.tensor_copy(
                            kT2[0:64, 2 * t + j, :],
                            kq_ps[0:64, 2 * j * 128 : (2 * j + 1) * 128],
                        )
                    nc.scalar.dma_start(
                        out=kT2[64:128, 2 * t : 2 * t + 2, :],
                        in_=kT2[0:64, 2 * t : 2 * t + 2, :],
                    )
                    # ---- attention q-block t ----
                    E = 2 * (t + 1)
                    av = av_pool.tile([H + 1, QB], f32, tag="av")
                    for s in range(t + 1):
                        c0, c1 = s, t + 1 + s
                        st0 = st_pool.tile([128, QB], f32, tag="st")
                        st1 = st_pool.tile([128, QB], f32, tag="st")
                        p = p_pool.tile([128, 2, QB], bf16, tag="p")
                        for hh, cid, stt in ((0, c0, st0), (1, c1, st1)):
                            nc.tensor.matmul(
                                stt[:],
                                lhsT=kT2[64 * hh : 64 * hh + 64, cid, :],
                                rhs=qT2[64 * hh : 64 * hh + 64, ts(t, QB)],
                                start=True,
                                stop=True,
                                tile_position=(64 * hh, 0),
                            )
                        # T0 chunk on ScalarE; T8 chunk on the DVE bit-trick,
                        # except every 6th slot (keeps the engines balanced)
                        nc.scalar.activation(
                            p[:, 0, :], st0[:], Act.Exp, scale=0.125
                        )
                        if set_idx % 6 == 5:
                            nc.scalar.activation(
                                p[:, 1, :], st1[:], Act.Exp, scale=0.125
                            )
                        else:
                            nc.vector.tensor_scalar(
                                p[:, 1, :].bitcast(u16),
                                st1[:],
                                SCH_A,
                                SCH_B,
                                Alu.mult,
                                Alu.add,
                            )
                        set_idx += 1
                        for hh, cid in ((0, c0), (1, c1)):
                            if cid >= E - 2:
                                nc.vector.tensor_tensor(
                                    p[:, hh, :],
                                    p[:, hh, :],
                                    mask_sb[:, cid - (E - 2), :],
                                    Alu.mult,
                                )
                        for hh, cid in ((0, c0), (1, c1)):
                            nc.tensor.matmul(
                                av[:],
                                lhsT=v_all[:, cid, 0 : H + 1],
                                rhs=p[:, hh, :],
                                start=(cid == 0),
                                stop=(cid == E - 1),
                            )
                    o = o_pool.tile([H + 1, QB], f32, tag="o")
                    nc.vector.tensor_copy(o[:], av[:])
                    nc.sync.dma_start(out=out_ext[:, ts(t, QB)], in_=o[:])

    nc.finalize()
    return nc


def _make_masks(g: int) -> np.ndarray:
    # mask[j][kk, qq]: qq is the PERMUTED block-local query col; its global
    # offset inside the block is qq_g. Key row kk belongs to own chunk with
    # in-block global offset 128*(2j+g).
    m = np.zeros((2, 128, QB), dtype=np.float32)
    qq = np.arange(QB)
    chunk4 = qq // 128
    if g == 1:
        chunk4 = chunk4 ^ 1
    qq_g = chunk4 * 128 + (qq % 128)
    for j in range(2):
        dk = 128 * (2 * j + g) + np.arange(128)[:, None]
        m[j] = (qq_g[None, :] >= dk).astype(np.float32)
    return m.astype(BF16)


def _perm_cols(a: np.ndarray, g: int) -> np.ndarray:
    """Pair-swap 128-col chunks along the last axis when g=1 (involution)."""
    if g == 0:
        return a
    shp = a.shape
    v = a.reshape(shp[:-1] + (shp[-1] // 256, 2, 128))
    return np.ascontiguousarray(v[..., ::-1, :].reshape(shp))


def _shard_inputs(input, Wq, Wk, Wv):
    x = np.asarray(input)
    wkq = np.concatenate([Wk, Wq], axis=1).astype(np.float32)  # [D, 128]
    wkq_h = np.ascontiguousarray(
        wkq.reshape(8, 128, 128).transpose(1, 0, 2).reshape(128, 8 * 128)
    ).astype(BF16)
    wv_h = np.ascontiguousarray(
        np.asarray(Wv, dtype=np.float32)
        .reshape(8, 128, H)
        .transpose(1, 0, 2)
        .reshape(128, 8 * H)
    ).astype(BF16)
    masks = []
    for g in range(2):
        m = _make_masks(g)  # [2, 128, QB]
        masks.append(np.ascontiguousarray(m.transpose(1, 0, 2).reshape(128, 2 * QB)))
    in_maps = []
    xTs = {}
    for b in range(B):
        xT = np.ascontiguousarray(x[b].T).astype(BF16)  # [D, S] global cols
        xTs[(b, 0)] = xT
        xTs[(b, 1)] = _perm_cols(xT, 1)
    for c in range(8):
        b, g = c // 2, c % 2
        in_maps.append(
            {"x": xTs[(b, g)], "wkq": wkq_h, "wv": wv_h, "mask": masks[g]}
        )
    return in_maps


def _unshard(results):
    out = np.empty((B, S, H), dtype=np.float32)
    for b in range(B):
        merged = results[2 * b]["out"] + _perm_cols(results[2 * b + 1]["out"], 1)
        out[b] = (merged[:H] / merged[H : H + 1]).T
    return out


def _run(inputs, trace=False):
    from concourse.bass_utils import run_bass_kernel_spmd

    if "nc" not in _CACHE:
        _CACHE["nc"] = _build()
    nc = _CACHE["nc"]
    in_maps = _shard_inputs(**inputs)
    res = run_bass_kernel_spmd(nc, in_maps, core_ids=list(range(8)), trace=trace)
    out = _unshard(res.results)
    return out, res


def kernel(**inputs) -> np.ndarray:
    out, _ = _run(inputs, trace=False)
    return out
